# revision 30
# baseline (speedup 1.0000x reference)
"""FourierBlock kernel for 8 TRN2 NeuronCores.

Math: the reference keeps only the first 64 rfft modes, so the whole op is
    out[b] = CS @ Y2[b],  Y2 = mode-mix(X2, W),  X2 = F2 @ q[b]
with F2 [128,4096] = [cos; -sin] forward-DFT rows and CS the inverse-DFT
columns (factor 2/L, except DC).  The complex combine (Yr = XrWr - XiWi etc.)
is folded into step-3's coefficient matrix CS4 [4096, 256] acting on the four
uncombined product groups (XrWr, XiWr, XrWi, XiWi).

Sharding: core c owns batch c for steps 1/3 (data parallel) and modes
[8c, 8c+8) for step 2 (tensor parallel over modes -> W is read exactly once
across the chip).  Two AllToAlls exchange the small X2 / product tensors.

Precision: step 1 runs in float32r (FP22), steps 2/3 in bf16 with fp32
accumulation (~3e-3 rel err total).  W is cast f32->bf16 during the DMA
(SWDGE) so the whole 8 MB bf16 W slice stays resident in SBUF -- the W
stream never stalls on pool slots and fully overlaps the first AllToAll.
"""

import numpy as np

B, L, D, M = 8, 4096, 512, 64
NCORES = 8
T = M // NCORES  # local modes per core


def _constants():
    import ml_dtypes

    k = np.arange(M)
    l = np.arange(L)
    ang = 2 * np.pi * np.outer(k, l) / L  # [M, L]
    # F2 row order (s, a, t): partition p = s*16 + a*8 + t holds
    # cos (a=0) / -sin (a=1) of mode k = 8s + t, so x2's partition layout
    # already equals the AllToAll bounce layout [s][a, t] (straight DMA).
    F2 = np.stack([np.cos(ang), -np.sin(ang)], axis=0)  # [2, M, L]
    F2 = F2.reshape(2, NCORES, T, L).transpose(1, 0, 2, 3).reshape(128, L)
    # lhsT chunks, p-major for contiguous DMA: fmat[p, n, m] = F2[m, n*128+p]
    fmat = np.ascontiguousarray(
        F2.T.reshape(32, 128, 128).transpose(1, 0, 2), dtype=np.float32
    )  # [128, 32, 128]

    ck = np.where(k == 0, 1.0, 2.0) / L
    ang2 = 2 * np.pi * np.outer(l, k) / L  # [L, M]
    C = (ck * np.cos(ang2)).reshape(L, NCORES, T)
    S = (-(2.0 / L) * np.sin(ang2)).reshape(L, NCORES, T)
    # K order (j, a, g2, t): (a0,g0)=rWr->C, (a0,g1)=rWi->S,
    # (a1,g0)=iWr->S, (a1,g1)=iWi->-C
    CS4 = np.empty((L, NCORES, 2, 2, T))
    CS4[:, :, 0, 0] = C
    CS4[:, :, 0, 1] = S
    CS4[:, :, 1, 0] = S
    CS4[:, :, 1, 1] = -C
    cmat = np.ascontiguousarray(
        CS4.reshape(L, 256).T.reshape(2, 128, L).astype(ml_dtypes.bfloat16)
    )  # [2, 128, L] bf16
    return fmat, cmat


def build_nc(debug=False):
    import concourse.bacc as bacc
    import concourse.mybir as mybir
    import concourse.tile as tile

    f32 = mybir.dt.float32
    f32r = mybir.dt.float32r
    bf16 = mybir.dt.bfloat16
    nc = bacc.Bacc("TRN2", target_bir_lowering=False, num_devices=NCORES)

    qb = nc.dram_tensor("qb", [L, D], f32r, kind="ExternalInput")
    # W pre-arranged on host: w[g2][p, t, dc, e] = W_g2[dc*128+p, e, 8c+t]
    wr = nc.dram_tensor("wr", [128, T, 4, 512], f32, kind="ExternalInput")
    wi = nc.dram_tensor("wi", [128, T, 4, 512], f32, kind="ExternalInput")
    out = nc.dram_tensor("out", [L, D], f32, kind="ExternalOutput")

    fmat_d = nc.dram_tensor("fmat", [128, 32, 128], f32r, kind="ExternalInput")
    cmat_d = nc.dram_tensor("cmat", [2, 128, L], bf16, kind="ExternalInput")
    ident_d = nc.dram_tensor("ident", [128, 128], f32r, kind="ExternalInput")
    if debug:
        dbg_x2 = nc.dram_tensor("dbg_x2", [128, 512], f32r, kind="ExternalOutput")
        dbg_xm = nc.dram_tensor("dbg_xm", [128, 512], f32r, kind="ExternalOutput")
        dbg_stage = nc.dram_tensor(
            "dbg_stage", [16, 2 * T * D], bf16, kind="ExternalOutput"
        )
        dbg_p = nc.dram_tensor("dbg_p", [2, 128, 512], bf16, kind="ExternalOutput")

    RG = [list(range(NCORES))]

    from concourse.tile_rust import add_dep_helper

    with tile.TileContext(nc) as tc:
        with (
            tc.tile_pool(name="constp", bufs=1) as constp,
            tc.tile_pool(name="qpool", bufs=3) as qpool,
            tc.tile_pool(name="wpool", bufs=1) as wpool,
            tc.tile_pool(name="misc", bufs=1) as misc,
            tc.tile_pool(name="outp", bufs=2) as outp,
            tc.tile_pool(name="pacc", bufs=3, space="PSUM") as pacc,
            tc.tile_pool(name="ptp", bufs=2, space="PSUM") as ptp,
            tc.tile_pool(name="po", bufs=3, space="PSUM") as po,
            tc.tile_pool(name="dram", bufs=1, space="DRAM") as dram,
        ):
            # constants (sync ring: fmat/ident first — step 1 needs them now)
            fmat_sb = constp.tile([128, 32 * 128], f32r)
            nc.sync.dma_start(
                out=fmat_sb[:].rearrange("p (n m) -> p n m", n=32), in_=fmat_d[:]
            )
            ident_sb = constp.tile([128, 128], f32r)
            nc.sync.dma_start(out=ident_sb[:], in_=ident_d[:])
            cmat_sb = constp.tile([128, 2 * L], bf16)
            cmat_dma = nc.scalar.dma_start(
                out=cmat_sb[:].rearrange("p (k m) -> p k m", k=2),
                in_=cmat_d[:].rearrange("k p m -> p k m"),
            )

            # ---- step 1 (f32r): X2 = F2 @ qb -> [128 (s,a,t), 512 d]
            x2ps = pacc.tile([128, 512], f32, tag="acc")
            last_q_dma = None
            for lo in range(8):  # 1 MB q transfers, 4 l-chunks each
                qt = qpool.tile([128, 4 * 512], f32r, name="qt", tag="qt")
                last_q_dma = nc.sync.dma_start(
                    out=qt[:].rearrange("p (n d) -> p n d", n=4),
                    in_=qb[:].rearrange("(n p) d -> p n d", p=128)[
                        :, lo * 4 : (lo + 1) * 4
                    ],
                )
                for li in range(4):
                    gl = lo * 4 + li
                    nc.tensor.matmul(
                        x2ps[:],
                        lhsT=fmat_sb[:, gl * 128 : (gl + 1) * 128],
                        rhs=qt[:, li * 512 : (li + 1) * 512],
                        start=(gl == 0),
                        stop=(gl == 31),
                    )

            # W: HWDGE f32 staging (scalar ring) + engine cast to resident
            # bf16.  HWDGE bulk traffic does not poison the ncfw mesh, so the
            # A2A can fire while W still streams.  First W DMA gated on the
            # last q load so q/fmat keep DMA priority.
            w_sb = []
            for g2 in range(2):
                wt = wpool.tile([128, T * 4 * 512], bf16, name=f"w{g2}")
                w_sb.append(wt)
            first = True
            for u in range(8):  # 2 modes per 2MB staged chunk
                g2, h = divmod(u, 4)
                wsrc = (wr, wi)[g2]
                wstg = qpool.tile([128, 4096], f32, name="wstg", tag="wstg", bufs=2)
                wdma = nc.scalar.dma_start(
                    out=wstg[:].rearrange("p (t n e) -> p t n e", t=2, n=4),
                    in_=wsrc[:, h * 2 : (h + 1) * 2],
                )
                if first:
                    first_w_dma = wdma
                    first = False
                nc.vector.tensor_copy(
                    w_sb[g2][:, h * 4096 : (h + 1) * 4096], wstg[:]
                )
            x2sb = misc.tile([128, 512], f32r)
            nc.scalar.copy(x2sb[:], x2ps[:])
            if debug:
                nc.sync.dma_start(out=dbg_x2[:], in_=x2sb[:])

            # ---- exchange 1: shard s = my batch's X rows for core s's modes
            b1in = dram.tile([NCORES, 2, T, D], f32r)
            nc.sync.dma_start(
                out=b1in[:].rearrange("s a t d -> (s a t) d"), in_=x2sb[:]
            )
            b1out = dram.tile([NCORES, 2, T, D], f32r)
            cc1 = nc.gpsimd.collective_compute(
                "AllToAll",
                mybir.AluOpType.bypass,
                replica_groups=RG,
                ins=[b1in.opt()],
                outs=[b1out.opt()],
            )
            # Bulk DMA on any ring delays the ncfw mesh start, so the first
            # AllToAll fires in a quiet system right after step 1 (absorbing
            # core skew early); the 16 MB W stream starts once it completes.
            add_dep_helper(
                first_w_dma.ins, cc1.ins, sync=True,
                reason="stream W only after the first AllToAll completes",
            )
            add_dep_helper(
                cmat_dma.ins, cc1.ins, sync=True,
                reason="defer cmat load out of the q/fmat critical stream",
            )

            # ---- load Xm [128 (j,a,t), 512 d], transpose, cast to bf16
            xm_sb = misc.tile([128, 512], f32r)
            nc.sync.dma_start(
                out=xm_sb[:], in_=b1out[:].rearrange("j a t d -> (j a t) d")
            )
            if debug:
                nc.sync.dma_start(out=dbg_xm[:], in_=xm_sb[:])
            xt_sb = misc.tile([128, 512], bf16)
            for dc in range(4):
                tp = ptp.tile([128, 128], f32r, name="tp", tag="tp")
                nc.tensor.transpose(
                    tp[:], xm_sb[:, dc * 128 : (dc + 1) * 128], ident_sb[:]
                )
                nc.scalar.copy(xt_sb[:, dc * 128 : (dc + 1) * 128], tp[:])

            # ---- step 2 (bf16): per (g2, t): products, M=16 cols m = 2j+a
            xt_v = xt_sb[:].rearrange("p (dc m t) -> p dc m t", dc=4, m=16, t=T)
            stage = misc.tile([16, 2 * T * D], bf16)  # free = (g2, t, e)
            for g2 in range(2):
                for t in range(T):
                    o = po.tile([16, 512], f32, name="o", tag="o")
                    for dc in range(4):
                        nc.tensor.matmul(
                            o[:],
                            lhsT=xt_v[:, dc, :, t],
                            rhs=w_sb[g2][
                                :, (t * 4 + dc) * 512 : (t * 4 + dc + 1) * 512
                            ],
                            start=(dc == 0),
                            stop=(dc == 3),
                        )
                    nc.scalar.copy(
                        stage[:, (g2 * T + t) * 512 : (g2 * T + t + 1) * 512], o[:]
                    )
            if debug:
                nc.sync.dma_start(out=dbg_stage[:], in_=stage[:])

            # ---- exchange 2: shard j = stage partitions 2j..2j+1, each
            # partition's (g2, t, e) contiguous
            b2in = dram.tile([NCORES, 2, 2, T, D], bf16)
            nc.sync.dma_start(
                out=b2in[:].rearrange("j a g2 t e -> (j a) (g2 t e)"), in_=stage[:]
            )
            b2out = dram.tile([NCORES, 2, 2, T, D], bf16)
            nc.gpsimd.collective_compute(
                "AllToAll",
                mybir.AluOpType.bypass,
                replica_groups=RG,
                ins=[b2in.opt()],
                outs=[b2out.opt()],
            )

            # ---- step 3 (bf16): out = CS4 @ P, K = 2 chunks (rows (j,a,g2,t))
            ps_rhs = []
            for kc in range(2):
                pg = pool_tile = misc.tile([128, 512], bf16, name=f"pg{kc}")
                nc.sync.dma_start(
                    out=pg[:],
                    in_=b2out[kc * 4 : (kc + 1) * 4].rearrange(
                        "j a g t e -> (j a g t) e"
                    ),
                )
                ps_rhs.append(pg)
            if debug:
                nc.sync.dma_start(out=dbg_p[0], in_=ps_rhs[0][:])
                nc.sync.dma_start(out=dbg_p[1], in_=ps_rhs[1][:])
            for mm in range(8):  # 4 m-chunks per 1 MB output DMA
                ot = outp.tile([128, 4 * 512], f32, name="ot", tag="ot")
                for sub in range(4):
                    m = mm * 4 + sub
                    ps = pacc.tile([128, 512], f32, name="ps3", tag="acc")
                    for kc in range(2):
                        nc.tensor.matmul(
                            ps[:],
                            lhsT=cmat_sb[
                                :, kc * L + m * 128 : kc * L + (m + 1) * 128
                            ],
                            rhs=ps_rhs[kc][:],
                            start=(kc == 0),
                            stop=(kc == 1),
                        )
                    cp = nc.scalar.copy if m % 2 else nc.vector.tensor_copy
                    cp(ot[:, sub * 512 : (sub + 1) * 512], ps[:])
                nc.sync.dma_start(
                    out=out[:].rearrange("(mm s p) e -> p mm s e", s=4, p=128)[
                        :, mm
                    ],
                    in_=ot[:].rearrange("p (s e) -> p s e", s=4),
                )

    nc.compile()
    return nc


_NC_CACHE = None


def _get_nc():
    global _NC_CACHE
    if _NC_CACHE is None:
        _NC_CACHE = build_nc()
    return _NC_CACHE


def _prep_w(w, sl):
    # [D, D, M] -> modes sl -> [128, T, 4, 512]: out[p, t, dc, e] = w[dc*128+p, e, t]
    wt = w[:, :, sl]  # [d, e, T]
    wt = wt.reshape(4, 128, 512, T).transpose(1, 3, 0, 2)
    return np.ascontiguousarray(wt, dtype=np.float32)


def run(q, w_real, w_imag, trace=False, debug=False):
    from concourse.bass_utils import run_bass_kernel_spmd

    nc = build_nc(debug=True) if debug else _get_nc()
    q = np.ascontiguousarray(np.asarray(q), dtype=np.float32)
    w_real = np.asarray(w_real)
    w_imag = np.asarray(w_imag)
    fmat_np, cmat_np = _constants()
    ident_np = np.eye(128, dtype=np.float32)
    in_maps = []
    for c in range(NCORES):
        sl = slice(c * T, (c + 1) * T)
        in_maps.append(
            {
                "qb": np.ascontiguousarray(q[c]),
                "wr": _prep_w(w_real, sl),
                "wi": _prep_w(w_imag, sl),
                "fmat": fmat_np,
                "cmat": cmat_np,
                "ident": ident_np,
            }
        )
    res = run_bass_kernel_spmd(
        nc, in_maps, core_ids=list(range(NCORES)), trace=trace
    )
    out = np.stack([r["out"] for r in res.results], axis=0)
    return out, res


def kernel(q, w_real, w_imag):
    out, _ = run(q, w_real, w_imag)
    return out


# revision 32
# speedup vs baseline: 1.0286x; 1.0286x over previous
"""FourierBlock kernel for 8 TRN2 NeuronCores.

Math: the reference keeps only the first 64 rfft modes, so the whole op is
    out[b] = CS @ Y2[b],  Y2 = mode-mix(X2, W),  X2 = F2 @ q[b]
with F2 [128,4096] = [cos; -sin] forward-DFT rows and CS the inverse-DFT
columns (factor 2/L, except DC).  The complex combine (Yr = XrWr - XiWi etc.)
is folded into step-3's coefficient matrix CS4 [4096, 256] acting on the four
uncombined product groups (XrWr, XiWr, XrWi, XiWi).

Sharding: core c owns batch c for steps 1/3 (data parallel) and modes
[8c, 8c+8) for step 2 (tensor parallel over modes -> W is read exactly once
across the chip).  Two AllToAlls exchange the small X2 / product tensors.

Precision: step 1 runs in float32r (FP22), steps 2/3 in bf16 with fp32
accumulation (~3e-3 rel err total).  W is cast f32->bf16 during the DMA
(SWDGE) so the whole 8 MB bf16 W slice stays resident in SBUF -- the W
stream never stalls on pool slots and fully overlaps the first AllToAll.
"""

import numpy as np

B, L, D, M = 8, 4096, 512, 64
NCORES = 8
T = M // NCORES  # local modes per core


def _constants():
    import ml_dtypes

    k = np.arange(M)
    l = np.arange(L)
    ang = 2 * np.pi * np.outer(k, l) / L  # [M, L]
    # F2 row order (s, a, t): partition p = s*16 + a*8 + t holds
    # cos (a=0) / -sin (a=1) of mode k = 8s + t, so x2's partition layout
    # already equals the AllToAll bounce layout [s][a, t] (straight DMA).
    F2 = np.stack([np.cos(ang), -np.sin(ang)], axis=0)  # [2, M, L]
    F2 = F2.reshape(2, NCORES, T, L).transpose(1, 0, 2, 3).reshape(128, L)
    # lhsT chunks, p-major for contiguous DMA: fmat[p, n, m] = F2[m, n*128+p]
    fmat = np.ascontiguousarray(
        F2.T.reshape(32, 128, 128).transpose(1, 0, 2), dtype=np.float32
    )  # [128, 32, 128]

    ck = np.where(k == 0, 1.0, 2.0) / L
    ang2 = 2 * np.pi * np.outer(l, k) / L  # [L, M]
    C = (ck * np.cos(ang2)).reshape(L, NCORES, T)
    S = (-(2.0 / L) * np.sin(ang2)).reshape(L, NCORES, T)
    # K order (j, a, g2, t): (a0,g0)=rWr->C, (a0,g1)=rWi->S,
    # (a1,g0)=iWr->S, (a1,g1)=iWi->-C
    CS4 = np.empty((L, NCORES, 2, 2, T))
    CS4[:, :, 0, 0] = C
    CS4[:, :, 0, 1] = S
    CS4[:, :, 1, 0] = S
    CS4[:, :, 1, 1] = -C
    cmat = np.ascontiguousarray(
        CS4.reshape(L, 256).T.reshape(2, 128, L).astype(ml_dtypes.bfloat16)
    )  # [2, 128, L] bf16
    return fmat, cmat


def build_nc(debug=False):
    import concourse.bacc as bacc
    import concourse.mybir as mybir
    import concourse.tile as tile

    f32 = mybir.dt.float32
    f32r = mybir.dt.float32r
    bf16 = mybir.dt.bfloat16
    nc = bacc.Bacc("TRN2", target_bir_lowering=False, num_devices=NCORES)

    qb = nc.dram_tensor("qb", [L, D], f32r, kind="ExternalInput")
    # W pre-arranged on host: w[g2][p, t, dc, e] = W_g2[dc*128+p, e, 8c+t]
    wr = nc.dram_tensor("wr", [128, T, 4, 512], f32, kind="ExternalInput")
    wi = nc.dram_tensor("wi", [128, T, 4, 512], f32, kind="ExternalInput")
    out = nc.dram_tensor("out", [L, D], f32, kind="ExternalOutput")

    fmat_d = nc.dram_tensor("fmat", [128, 32, 128], f32r, kind="ExternalInput")
    cmat_d = nc.dram_tensor("cmat", [2, 128, L], bf16, kind="ExternalInput")
    ident_d = nc.dram_tensor("ident", [128, 128], f32r, kind="ExternalInput")
    if debug:
        dbg_x2 = nc.dram_tensor("dbg_x2", [128, 512], f32r, kind="ExternalOutput")
        dbg_xm = nc.dram_tensor("dbg_xm", [128, 512], f32r, kind="ExternalOutput")
        dbg_stage = nc.dram_tensor(
            "dbg_stage", [16, 2 * T * D], bf16, kind="ExternalOutput"
        )
        dbg_p = nc.dram_tensor("dbg_p", [2, 128, 512], bf16, kind="ExternalOutput")

    RG = [list(range(NCORES))]

    from concourse.tile_rust import add_dep_helper

    with tile.TileContext(nc) as tc:
        with (
            tc.tile_pool(name="constp", bufs=1) as constp,
            tc.tile_pool(name="qpool", bufs=3) as qpool,
            tc.tile_pool(name="wpool", bufs=1) as wpool,
            tc.tile_pool(name="misc", bufs=1) as misc,
            tc.tile_pool(name="outp", bufs=3) as outp,
            tc.tile_pool(name="pacc", bufs=3, space="PSUM") as pacc,
            tc.tile_pool(name="ptp", bufs=2, space="PSUM") as ptp,
            tc.tile_pool(name="po", bufs=3, space="PSUM") as po,
            tc.tile_pool(name="dram", bufs=1, space="DRAM") as dram,
        ):
            # constants (sync ring: fmat/ident first — step 1 needs them now)
            fmat_sb = constp.tile([128, 32 * 128], f32r)
            nc.sync.dma_start(
                out=fmat_sb[:].rearrange("p (n m) -> p n m", n=32), in_=fmat_d[:]
            )
            ident_sb = constp.tile([128, 128], f32r)
            nc.sync.dma_start(out=ident_sb[:], in_=ident_d[:])
            cmat_sb = constp.tile([128, 2 * L], bf16)
            cmat_dma = nc.scalar.dma_start(
                out=cmat_sb[:].rearrange("p (k m) -> p k m", k=2),
                in_=cmat_d[:].rearrange("k p m -> p k m"),
            )

            # ---- step 1 (f32r): X2 = F2 @ qb -> [128 (s,a,t), 512 d]
            x2ps = pacc.tile([128, 512], f32, tag="acc")
            last_q_dma = None
            for lo in range(8):  # 1 MB q transfers, 4 l-chunks each
                qt = qpool.tile([128, 4 * 512], f32r, name="qt", tag="qt")
                last_q_dma = nc.sync.dma_start(
                    out=qt[:].rearrange("p (n d) -> p n d", n=4),
                    in_=qb[:].rearrange("(n p) d -> p n d", p=128)[
                        :, lo * 4 : (lo + 1) * 4
                    ],
                )
                for li in range(4):
                    gl = lo * 4 + li
                    nc.tensor.matmul(
                        x2ps[:],
                        lhsT=fmat_sb[:, gl * 128 : (gl + 1) * 128],
                        rhs=qt[:, li * 512 : (li + 1) * 512],
                        start=(gl == 0),
                        stop=(gl == 31),
                    )

            # W: HWDGE f32 staging (scalar ring) + engine cast to resident
            # bf16.  HWDGE bulk traffic does not poison the ncfw mesh, so the
            # A2A can fire while W still streams.  First W DMA gated on the
            # last q load so q/fmat keep DMA priority.
            w_sb = []
            for g2 in range(2):
                wt = wpool.tile([128, T * 4 * 512], bf16, name=f"w{g2}")
                w_sb.append(wt)
            first = True
            for u in range(8):  # 2 modes per 2MB staged chunk
                g2, h = divmod(u, 4)
                wsrc = (wr, wi)[g2]
                wstg = qpool.tile([128, 4096], f32, name="wstg", tag="wstg")
                wdma = nc.scalar.dma_start(
                    out=wstg[:].rearrange("p (t n e) -> p t n e", t=2, n=4),
                    in_=wsrc[:, h * 2 : (h + 1) * 2],
                )
                if first:
                    first_w_dma = wdma
                    first = False
                nc.vector.tensor_copy(
                    w_sb[g2][:, h * 4096 : (h + 1) * 4096], wstg[:]
                )
            x2sb = misc.tile([128, 512], f32r)
            nc.scalar.copy(x2sb[:], x2ps[:])
            if debug:
                nc.sync.dma_start(out=dbg_x2[:], in_=x2sb[:])

            # ---- exchange 1: shard s = my batch's X rows for core s's modes
            b1in = dram.tile([NCORES, 2, T, D], f32r)
            nc.sync.dma_start(
                out=b1in[:].rearrange("s a t d -> (s a t) d"), in_=x2sb[:]
            )
            # Keep the PE clock warm through the A2A1 stall (HAM re-throttles
            # to 1.2 GHz after ~3.4us idle; step 2 would start cold).  Anchored
            # on x2sb so these run exactly during the collective window.
            warm1 = pacc.tile([128, 512], f32, name="warm1", tag="acc")
            for i in range(64):
                nc.tensor.matmul(
                    warm1[:], lhsT=x2sb[:, 0:128], rhs=x2sb[:],
                    start=(i == 0), stop=(i == 63),
                )
            b1out = dram.tile([NCORES, 2, T, D], f32r)
            cc1 = nc.gpsimd.collective_compute(
                "AllToAll",
                mybir.AluOpType.bypass,
                replica_groups=RG,
                ins=[b1in.opt()],
                outs=[b1out.opt()],
            )
            # Bulk DMA on any ring delays the ncfw mesh start, so the first
            # AllToAll fires in a quiet system right after step 1 (absorbing
            # core skew early); the 16 MB W stream starts once it completes.
            add_dep_helper(
                first_w_dma.ins, cc1.ins, sync=True,
                reason="stream W only after the first AllToAll completes",
            )
            add_dep_helper(
                cmat_dma.ins, cc1.ins, sync=True,
                reason="defer cmat load out of the q/fmat critical stream",
            )

            # ---- load Xm [128 (j,a,t), 512 d], transpose, cast to bf16
            xm_sb = misc.tile([128, 512], f32r)
            nc.sync.dma_start(
                out=xm_sb[:], in_=b1out[:].rearrange("j a t d -> (j a t) d")
            )
            if debug:
                nc.sync.dma_start(out=dbg_xm[:], in_=xm_sb[:])
            xt_sb = misc.tile([128, 512], bf16)
            for dc in range(4):
                tp = ptp.tile([128, 128], f32r, name="tp", tag="tp")
                nc.tensor.transpose(
                    tp[:], xm_sb[:, dc * 128 : (dc + 1) * 128], ident_sb[:]
                )
                nc.scalar.copy(xt_sb[:, dc * 128 : (dc + 1) * 128], tp[:])

            # ---- step 2 (bf16): per (g2, t): products, M=16 cols m = 2j+a
            xt_v = xt_sb[:].rearrange("p (dc m t) -> p dc m t", dc=4, m=16, t=T)
            stage = misc.tile([16, 2 * T * D], bf16)  # free = (g2, t, e)
            for g2 in range(2):
                for t in range(T):
                    o = po.tile([16, 512], f32, name="o", tag="o")
                    for dc in range(4):
                        nc.tensor.matmul(
                            o[:],
                            lhsT=xt_v[:, dc, :, t],
                            rhs=w_sb[g2][
                                :, (t * 4 + dc) * 512 : (t * 4 + dc + 1) * 512
                            ],
                            start=(dc == 0),
                            stop=(dc == 3),
                        )
                    nc.scalar.copy(
                        stage[:, (g2 * T + t) * 512 : (g2 * T + t + 1) * 512], o[:]
                    )
            if debug:
                nc.sync.dma_start(out=dbg_stage[:], in_=stage[:])

            # ---- exchange 2: shard j = stage partitions 2j..2j+1, each
            # partition's (g2, t, e) contiguous
            b2in = dram.tile([NCORES, 2, 2, T, D], bf16)
            nc.sync.dma_start(
                out=b2in[:].rearrange("j a g2 t e -> (j a) (g2 t e)"), in_=stage[:]
            )
            # Same trick for the A2A2 window: anchored on the last-written
            # stage slice so these run during the second collective.
            warm2 = pacc.tile([128, 512], f32, name="warm2", tag="acc")
            for i in range(64):
                nc.tensor.matmul(
                    warm2[:],
                    lhsT=stage[:, 15 * 512 : 15 * 512 + 128],
                    rhs=stage[:, 15 * 512 : 16 * 512],
                    start=(i == 0), stop=(i == 63),
                )
            b2out = dram.tile([NCORES, 2, 2, T, D], bf16)
            nc.gpsimd.collective_compute(
                "AllToAll",
                mybir.AluOpType.bypass,
                replica_groups=RG,
                ins=[b2in.opt()],
                outs=[b2out.opt()],
            )

            # ---- step 3 (bf16): out = CS4 @ P, K = 2 chunks (rows (j,a,g2,t))
            ps_rhs = []
            for kc in range(2):
                pg = pool_tile = misc.tile([128, 512], bf16, name=f"pg{kc}")
                nc.sync.dma_start(
                    out=pg[:],
                    in_=b2out[kc * 4 : (kc + 1) * 4].rearrange(
                        "j a g t e -> (j a g t) e"
                    ),
                )
                ps_rhs.append(pg)
            if debug:
                nc.sync.dma_start(out=dbg_p[0], in_=ps_rhs[0][:])
                nc.sync.dma_start(out=dbg_p[1], in_=ps_rhs[1][:])
            for m in range(32):
                ps = pacc.tile([128, 512], f32, name="ps3", tag="acc")
                for kc in range(2):
                    nc.tensor.matmul(
                        ps[:],
                        lhsT=cmat_sb[:, kc * L + m * 128 : kc * L + (m + 1) * 128],
                        rhs=ps_rhs[kc][:],
                        start=(kc == 0),
                        stop=(kc == 1),
                    )
                ot = outp.tile([128, 512], f32, name="ot", tag="ot")
                cp = nc.scalar.copy if m % 2 else nc.vector.tensor_copy
                cp(ot[:], ps[:])
                nc.sync.dma_start(out=out[m * 128 : (m + 1) * 128, :], in_=ot[:])

    nc.compile()
    return nc


_NC_CACHE = None


def _get_nc():
    global _NC_CACHE
    if _NC_CACHE is None:
        _NC_CACHE = build_nc()
    return _NC_CACHE


def _prep_w(w, sl):
    # [D, D, M] -> modes sl -> [128, T, 4, 512]: out[p, t, dc, e] = w[dc*128+p, e, t]
    wt = w[:, :, sl]  # [d, e, T]
    wt = wt.reshape(4, 128, 512, T).transpose(1, 3, 0, 2)
    return np.ascontiguousarray(wt, dtype=np.float32)


def run(q, w_real, w_imag, trace=False, debug=False):
    from concourse.bass_utils import run_bass_kernel_spmd

    nc = build_nc(debug=True) if debug else _get_nc()
    q = np.ascontiguousarray(np.asarray(q), dtype=np.float32)
    w_real = np.asarray(w_real)
    w_imag = np.asarray(w_imag)
    fmat_np, cmat_np = _constants()
    ident_np = np.eye(128, dtype=np.float32)
    in_maps = []
    for c in range(NCORES):
        sl = slice(c * T, (c + 1) * T)
        in_maps.append(
            {
                "qb": np.ascontiguousarray(q[c]),
                "wr": _prep_w(w_real, sl),
                "wi": _prep_w(w_imag, sl),
                "fmat": fmat_np,
                "cmat": cmat_np,
                "ident": ident_np,
            }
        )
    res = run_bass_kernel_spmd(
        nc, in_maps, core_ids=list(range(NCORES)), trace=trace
    )
    out = np.stack([r["out"] for r in res.results], axis=0)
    return out, res


def kernel(q, w_real, w_imag):
    out, _ = run(q, w_real, w_imag)
    return out
